# revision 12
# baseline (speedup 1.0000x reference)
"""Contrastive-loss Trainium2 kernel: 8-way data-parallel over similarity rows.

Strategy (per sharding hint): each of the 8 NeuronCores computes a
[1024, 8192] block of the similarity matrix sim = e @ e.T / T against the
full embedding matrix, reduces per-row numerator / denominator / validity
on-device, and returns per-partition partial (loss_sum, valid_count); the
host sums the 8x[128,2] partials.

Key layout trick: rows are sorted by label on the host and each core's
input is rolled so its 1024 rows sit at a fixed offset (PAD). Same-label
columns of any 128-row tile then live in a fixed 640-wide window
[t*128, t*128+640), so the label-mask / positive-gate / numerator work
touches 640 instead of 8192 columns per row. The denominator row-sum comes
free from the Exp activation's accum_out. Matmuls run in bf16 (fp32 PSUM
accumulate); everything downstream of exp is fp32.
"""

import contextlib
import ctypes
import os
import sys
import types

import ml_dtypes
import numpy as np

import concourse.bass as bass
import concourse.mybir as mybir
import concourse.tile as tile
from concourse.bass_utils import run_bass_kernel_spmd

# problem constants (hardcoded per task contract)
N, D, NCLS = 8192, 512, 512
TEMP = 0.07
EPS = 1e-8
M = 8            # cores
R = N // M       # 1024 rows per core
NT = R // 128    # 8 row-tiles per core
PAD = 256        # roll margin; must exceed max class size
WIN = 128 + 2 * PAD   # 640 col window containing all same-label cols of a tile
CH = 512         # matmul moving-dim chunk (one PSUM bank)
GRP = 1024       # columns per psum group / exp call (2 banks)
NG = N // GRP    # 4 groups
KT = D // 128    # 4 contraction tiles

_AXON_SO = "/opt/axon/libaxon_pjrt.so"

LAST_RESULTS = None   # BassKernelResults of the most recent run (for test.py)


def _install_axon_trace_hook():
    """Provide antenv.axon_hooks (NTFF profiling) if the image lacks it."""
    try:
        from antenv.axon_hooks import get_axon_ntff_profile_hook  # noqa: F401
        return
    except ImportError:
        pass
    if not os.path.exists(_AXON_SO):
        return
    try:
        lib = ctypes.CDLL(_AXON_SO)
    except OSError:
        return
    if not hasattr(lib, "axon_start_nrt_profile"):
        return
    lib.axon_start_nrt_profile.argtypes = [ctypes.POINTER(ctypes.c_int64), ctypes.c_size_t]
    lib.axon_start_nrt_profile.restype = ctypes.c_int64
    lib.axon_stop_nrt_profile.argtypes = [ctypes.c_char_p]
    lib.axon_stop_nrt_profile.restype = ctypes.c_int64

    @contextlib.contextmanager
    def _hook(output_dir, device_ids):
        import jax
        jax.devices()
        if device_ids:
            ids = (ctypes.c_int64 * len(device_ids))(*device_ids)
            rc = lib.axon_start_nrt_profile(ids, len(device_ids))
        else:
            rc = lib.axon_start_nrt_profile(None, 0)
        if rc != 0:
            raise RuntimeError(f"axon_start_nrt_profile rc={rc}")
        try:
            yield
        finally:
            n = lib.axon_stop_nrt_profile(str(output_dir).encode())
            if n < 0:
                raise RuntimeError(f"axon_stop_nrt_profile rc={n}")

    _the_hook = [_hook]
    mod = types.ModuleType("antenv.axon_hooks")
    mod.set_axon_ntff_profile_hook = lambda h: _the_hook.__setitem__(0, h)
    mod.get_axon_ntff_profile_hook = lambda: _the_hook[0]
    sys.modules["antenv.axon_hooks"] = mod
    import antenv
    antenv.axon_hooks = mod


def _split_excess_waits(nc, max_waits=1):
    """This walrus build allows one sync-wait per instruction; move extras
    onto same-engine NoOps inserted just before (execution order preserved)."""
    for f in nc.m.functions:
        for b in f.blocks:
            insts = b.instructions
            new = []
            changed = False
            for inst in insts:
                si = inst.sync_info
                ow = list(si.on_wait) if (si and si.on_wait) else []
                if len(ow) > max_waits:
                    extra, keep = ow[:-max_waits], ow[-max_waits:]
                    for k, w in enumerate(extra):
                        nop = mybir.InstNoOp(name=f"{inst.name}-w{k}", ins=[], outs=[])
                        nop.engine = inst.engine
                        nop.sync_info = mybir.SyncInfo(on_wait=[w], on_update=[])
                        new.append(nop)
                    inst.sync_info = mybir.SyncInfo(
                        on_wait=keep,
                        on_update=list(si.on_update) if si.on_update else [])
                    changed = True
                new.append(inst)
            if changed:
                b.instructions = new


def _build_nc():
    f32 = mybir.dt.float32
    bf16 = mybir.dt.bfloat16
    Alu = mybir.AluOpType
    Act = mybir.ActivationFunctionType

    nc = bass.Bass(trn_type="TRN2", target_bir_lowering=False, debug=False)
    qT = nc.dram_tensor("qT", [128, (N // CH) * KT * CH], bf16, kind="ExternalInput")
    labd = nc.dram_tensor("lab", [N, 1], f32, kind="ExternalInput")
    antid = nc.dram_tensor("anti", [128, 128], f32, kind="ExternalInput")
    identd = nc.dram_tensor("ident", [128, 128], f32, kind="ExternalInput")
    outd = nc.dram_tensor("out", [128, 2], f32, kind="ExternalOutput")

    with tile.TileContext(nc) as tc, contextlib.ExitStack() as ctx:
        qp = ctx.enter_context(tc.tile_pool(name="qp", bufs=1))
        pp = ctx.enter_context(tc.tile_pool(name="pp", bufs=4, space="PSUM"))
        ep = ctx.enter_context(tc.tile_pool(name="ep", bufs=3))
        wp = ctx.enter_context(tc.tile_pool(name="wp", bufs=2))
        sp = ctx.enter_context(tc.tile_pool(name="sp", bufs=1))

        # ---- preload ----
        # qT chunks: [128, KT, CH] bf16, one per 512-col chunk
        q0k = []
        for k in range(KT):
            q = qp.tile([128, CH], bf16, tag=f"q0k{k}")
            nc.sync.dma_start(out=q, in_=qT[:, k * CH:(k + 1) * CH])
            q0k.append(q)
        qt = [None]
        for n in range(1, N // CH):
            q = qp.tile([128, KT, CH], bf16, tag=f"q{n}")
            nc.sync.dma_start(
                out=q, in_=qT[:, n * KT * CH:(n + 1) * KT * CH])
            qt.append(q)

        def qslice(n, k, lo=0, hi=CH):
            if n == 0:
                return q0k[k][:, lo:hi]
            return qt[n][:, k, lo:hi]
        # row labels per (partition, tile): lab[PAD + t*128 + p]
        lab_rows = sp.tile([128, NT, 1], f32)
        nc.sync.dma_start(
            out=lab_rows,
            in_=labd[PAD:PAD + R, :].rearrange("(t p) o -> p t o", p=128))
        # column labels broadcast to all partitions, cols [0, NT*128+WIN)
        labw_w = (NT - 1) * 128 + WIN        # 1536
        labw = sp.tile([128, labw_w], f32)
        nc.sync.dma_start(
            out=labw,
            in_=bass.AP(tensor=labd, offset=0, ap=[[0, 128], [1, labw_w]]))
        anti = sp.tile([128, 128], f32)
        nc.sync.dma_start(out=anti, in_=antid.ap())
        ident = sp.tile([128, 128], f32)
        nc.sync.dma_start(out=ident, in_=identd.ap())
        eps_t = sp.tile([128, 1], f32)
        nc.vector.memset(eps_t, EPS)
        warm = sp.tile([128, 128], bf16)
        nc.vector.memset(warm, 0.0)
        warm_ps = pp.tile([128, GRP], f32, tag="ps")
        for w in range(48):
            nc.tensor.matmul(warm_ps[:, :128], warm, warm, start=True, stop=True)

        # ---- accumulators ----
        dacc = sp.tile([128, NT * NG], f32)   # exp row-sums per (t, g)
        nacc = sp.tile([128, NT * 2], f32)    # numerator per (t, half)
        nc.vector.memset(nacc, 0.0)
        edacc = sp.tile([128, NT], f32)       # diagonal exp per t

        # ---- main loop ----
        for t in range(NT):
            a = (PAD + t * 128) // CH        # lhsT chunk index
            off = (PAD + t * 128) % CH       # lhsT offset within chunk
            for g in range(NG):
                ps = pp.tile([128, GRP], f32, tag="ps")
                for k in range(KT):
                    for sub in range(GRP // CH):
                        n = g * (GRP // CH) + sub
                        nc.tensor.matmul(
                            ps[:, sub * CH:(sub + 1) * CH],
                            qslice(a, k, off, off + 128),
                            qslice(n, k),
                            start=(k == 0), stop=(k == KT - 1))
                e = ep.tile([128, GRP], f32, tag="e")
                nc.scalar.activation(
                    out=e, in_=ps[:], func=Act.Exp, scale=float(1.0 / TEMP),
                    accum_out=dacc[:, t * NG + g:t * NG + g + 1])
                # window = cols [t*128, t*128+WIN) may span groups g, g+1
                wlo, whi = t * 128, t * 128 + WIN
                glo, ghi = g * GRP, (g + 1) * GRP
                lo, hi = max(wlo, glo), min(whi, ghi)
                if lo < hi:
                    L = hi - lo
                    widx = 0 if lo == wlo else 1
                    u = wp.tile([128, WIN], f32, tag="u")
                    # u = (lab_col == lab_row) * exp(sim)
                    nc.vector.scalar_tensor_tensor(
                        out=u[:, :L], in0=labw[:, lo:hi],
                        scalar=lab_rows[:, t, :], in1=e[:, lo - glo:hi - glo],
                        op0=Alu.is_equal, op1=Alu.mult)
                    # diagonal sits at global cols [PAD+t*128, PAD+t*128+128)
                    dlo = PAD + t * 128
                    if lo <= dlo and dlo + 128 <= hi:
                        dl = dlo - lo
                        scr = wp.tile([128, 128], f32, tag="scr")
                        nc.vector.scalar_tensor_tensor(
                            out=scr, in0=u[:, dl:dl + 128], scalar=1.0,
                            in1=ident, op0=Alu.mult, op1=Alu.mult,
                            accum_out=edacc[:, t:t + 1])
                        nc.vector.tensor_tensor(
                            out=u[:, dl:dl + 128], in0=u[:, dl:dl + 128],
                            in1=anti, op=Alu.mult)
                    # numerator: sum over u where u > 1  (sim>0 gate)
                    scr2 = wp.tile([128, WIN], f32, tag="scr2")
                    nc.vector.scalar_tensor_tensor(
                        out=scr2[:, :L], in0=u[:, :L], scalar=1.0, in1=u[:, :L],
                        op0=Alu.is_gt, op1=Alu.mult,
                        accum_out=nacc[:, 2 * t + widx:2 * t + widx + 1])

        # ---- epilogue (all [128, NT]) ----
        dred = sp.tile([128, NT], f32)
        nc.vector.tensor_reduce(
            out=dred, in_=dacc.rearrange("p (t g) -> p t g", g=NG),
            axis=mybir.AxisListType.X, op=Alu.add)
        den = sp.tile([128, NT], f32)
        nc.vector.tensor_tensor(out=den, in0=dred, in1=edacc, op=Alu.subtract)
        numr = sp.tile([128, NT], f32)
        nc.vector.tensor_reduce(
            out=numr, in_=nacc.rearrange("p (t w) -> p t w", w=2),
            axis=mybir.AxisListType.X, op=Alu.add)
        v1 = sp.tile([128, NT], f32)
        nc.vector.tensor_scalar(out=v1, in0=numr, scalar1=0.0, scalar2=None,
                                op0=Alu.is_gt)
        v2 = sp.tile([128, NT], f32)
        nc.vector.tensor_scalar(out=v2, in0=den, scalar1=0.0, scalar2=None,
                                op0=Alu.is_gt)
        v = sp.tile([128, NT], f32)
        nc.vector.tensor_tensor(out=v, in0=v1, in1=v2, op=Alu.mult)
        inv = sp.tile([128, NT], f32)
        nc.vector.tensor_scalar(out=inv, in0=v, scalar1=0.0, scalar2=None,
                                op0=Alu.is_equal)
        nsafe = sp.tile([128, NT], f32)
        nc.vector.tensor_tensor(out=nsafe, in0=numr, in1=v, op=Alu.mult)
        nc.vector.tensor_tensor(out=nsafe, in0=nsafe, in1=inv, op=Alu.add)
        dsafe = sp.tile([128, NT], f32)
        nc.vector.tensor_tensor(out=dsafe, in0=den, in1=v, op=Alu.mult)
        nc.vector.tensor_tensor(out=dsafe, in0=dsafe, in1=inv, op=Alu.add)
        lgd = sp.tile([128, NT], f32)
        nc.scalar.activation(out=lgd, in_=dsafe, func=Act.Ln, bias=eps_t[:], scale=1.0)
        lgn = sp.tile([128, NT], f32)
        nc.scalar.activation(out=lgn, in_=nsafe, func=Act.Ln, scale=1.0)
        li = sp.tile([128, NT], f32)
        nc.vector.tensor_tensor(out=li, in0=lgd, in1=lgn, op=Alu.subtract)
        nc.vector.tensor_tensor(out=li, in0=li, in1=v, op=Alu.mult)
        o = sp.tile([128, 2], f32)
        nc.vector.tensor_reduce(out=o[:, 0:1], in_=li, axis=mybir.AxisListType.X,
                                op=Alu.add)
        nc.vector.tensor_reduce(out=o[:, 1:2], in_=v, axis=mybir.AxisListType.X,
                                op=Alu.add)
        nc.sync.dma_start(out=outd.ap(), in_=o)

    _split_excess_waits(nc)
    return nc


_NC = None


def _get_nc():
    global _NC
    if _NC is None:
        _NC = _build_nc()
    return _NC


def _host_reference(emb, lab):
    """Numpy fallback (only for pathological label distributions where a
    class exceeds the PAD margin; never triggers for the target regime)."""
    e = emb / np.linalg.norm(emb, axis=1, keepdims=True).astype(np.float32)
    sim = (e @ e.T).astype(np.float32) / np.float32(TEMP)
    E = np.exp(sim, dtype=np.float32)
    pos = (lab[:, None] == lab[None, :]) & ~np.eye(len(lab), dtype=bool)
    valid = pos & (sim > 0)
    num = np.where(valid, E, 0).sum(1, dtype=np.float32)
    den = E.sum(1, dtype=np.float32) - np.diagonal(E)
    rv = valid.any(1) & (den > 0)
    ns = np.where(rv, num, np.float32(1.0))
    ds = np.where(rv, den, np.float32(1.0))
    li = np.log(ds + np.float32(EPS)) - np.log(ns)
    nv = int(rv.sum())
    if nv == 0:
        return np.float32(0.0)
    return np.float32(abs(float(np.where(rv, li, 0).sum(dtype=np.float64)) / nv))


def kernel(**inputs):
    global LAST_RESULTS
    emb = np.ascontiguousarray(np.asarray(inputs["embeddings"], dtype=np.float32))
    lab = np.asarray(inputs["labels"]).astype(np.int64).ravel()
    assert emb.shape == (N, D) and lab.shape == (N,)

    if np.bincount(lab, minlength=1).max() > PAD:
        return _host_reference(emb, lab)

    _install_axon_trace_hook()

    # host prep: normalize, sort by label, per-core roll + transpose
    e = emb / np.linalg.norm(emb, axis=1, keepdims=True).astype(np.float32)
    order = np.argsort(lab, kind="stable")
    es = np.ascontiguousarray(e[order])
    ls = lab[order].astype(np.float32)

    anti = (1.0 - np.eye(128, dtype=np.float32)).astype(np.float32)
    ident = np.eye(128, dtype=np.float32)

    in_maps = []
    for c in range(M):
        shift = c * R - PAD
        rolled = np.roll(es, -shift, axis=0)         # [N, D] f32
        labr = np.roll(ls, -shift).reshape(N, 1)     # [N, 1] f32
        # [D, N] -> [128, NCH, KT, CH]: partition p, chunk n holds
        # qT[k*128+p, n*CH:(n+1)*CH] contiguckus per (k)
        qTc = (rolled.T.reshape(KT, 128, N // CH, CH)
               .transpose(1, 2, 0, 3)
               .reshape(128, (N // CH) * KT * CH)
               .astype(ml_dtypes.bfloat16))
        qTc = np.ascontiguousarray(qTc)
        in_maps.append({
            "qT": qTc,
            "lab": np.ascontiguousarray(labr),
            "anti": anti,
            "ident": ident,
        })

    nc = _get_nc()
    res = run_bass_kernel_spmd(nc, in_maps, core_ids=list(range(M)))
    LAST_RESULTS = res

    loss_sum = 0.0
    cnt = 0.0
    for c in range(M):
        o = res.results[c]["out"]
        loss_sum += float(o[:, 0].sum(dtype=np.float64))
        cnt += float(o[:, 1].sum(dtype=np.float64))
    if cnt <= 0:
        return np.float32(0.0)
    return np.float32(abs(loss_sum / cnt))


# revision 13
# speedup vs baseline: 1.0071x; 1.0071x over previous
"""Contrastive-loss Trainium2 kernel: 8-way data-parallel over similarity rows.

Strategy (per sharding hint): each of the 8 NeuronCores computes a
[1024, 8192] block of the similarity matrix sim = e @ e.T / T against the
full embedding matrix, reduces per-row numerator / denominator / validity
on-device, and returns per-partition partial (loss_sum, valid_count); the
host sums the 8x[128,2] partials.

Key layout trick: rows are sorted by label on the host and each core's
input is rolled so its 1024 rows sit at a fixed offset (PAD). Same-label
columns of any 128-row tile then live in a fixed 640-wide window
[t*128, t*128+640), so the label-mask / positive-gate / numerator work
touches 640 instead of 8192 columns per row. The denominator row-sum comes
free from the Exp activation's accum_out. Matmuls run in bf16 (fp32 PSUM
accumulate); everything downstream of exp is fp32.
"""

import contextlib
import ctypes
import os
import sys
import types

import ml_dtypes
import numpy as np

import concourse.bass as bass
import concourse.mybir as mybir
import concourse.tile as tile
from concourse.bass_utils import run_bass_kernel_spmd

# problem constants (hardcoded per task contract)
N, D, NCLS = 8192, 512, 512
TEMP = 0.07
EPS = 1e-8
M = 8            # cores
R = N // M       # 1024 rows per core
NT = R // 128    # 8 row-tiles per core
PAD = 256        # roll margin; must exceed max class size
WIN = 128 + 2 * PAD   # 640 col window containing all same-label cols of a tile
CH = 512         # matmul moving-dim chunk (one PSUM bank)
GRP = 2048       # columns per psum group / exp call (4 banks)
NG = N // GRP    # 4 groups
KT = D // 128    # 4 contraction tiles

_AXON_SO = "/opt/axon/libaxon_pjrt.so"

LAST_RESULTS = None   # BassKernelResults of the most recent run (for test.py)


def _install_axon_trace_hook():
    """Provide antenv.axon_hooks (NTFF profiling) if the image lacks it."""
    try:
        from antenv.axon_hooks import get_axon_ntff_profile_hook  # noqa: F401
        return
    except ImportError:
        pass
    if not os.path.exists(_AXON_SO):
        return
    try:
        lib = ctypes.CDLL(_AXON_SO)
    except OSError:
        return
    if not hasattr(lib, "axon_start_nrt_profile"):
        return
    lib.axon_start_nrt_profile.argtypes = [ctypes.POINTER(ctypes.c_int64), ctypes.c_size_t]
    lib.axon_start_nrt_profile.restype = ctypes.c_int64
    lib.axon_stop_nrt_profile.argtypes = [ctypes.c_char_p]
    lib.axon_stop_nrt_profile.restype = ctypes.c_int64

    @contextlib.contextmanager
    def _hook(output_dir, device_ids):
        import jax
        jax.devices()
        if device_ids:
            ids = (ctypes.c_int64 * len(device_ids))(*device_ids)
            rc = lib.axon_start_nrt_profile(ids, len(device_ids))
        else:
            rc = lib.axon_start_nrt_profile(None, 0)
        if rc != 0:
            raise RuntimeError(f"axon_start_nrt_profile rc={rc}")
        try:
            yield
        finally:
            n = lib.axon_stop_nrt_profile(str(output_dir).encode())
            if n < 0:
                raise RuntimeError(f"axon_stop_nrt_profile rc={n}")

    _the_hook = [_hook]
    mod = types.ModuleType("antenv.axon_hooks")
    mod.set_axon_ntff_profile_hook = lambda h: _the_hook.__setitem__(0, h)
    mod.get_axon_ntff_profile_hook = lambda: _the_hook[0]
    sys.modules["antenv.axon_hooks"] = mod
    import antenv
    antenv.axon_hooks = mod


def _split_excess_waits(nc, max_waits=1):
    """This walrus build allows one sync-wait per instruction; move extras
    onto same-engine NoOps inserted just before (execution order preserved)."""
    for f in nc.m.functions:
        for b in f.blocks:
            insts = b.instructions
            new = []
            changed = False
            for inst in insts:
                si = inst.sync_info
                ow = list(si.on_wait) if (si and si.on_wait) else []
                if len(ow) > max_waits:
                    extra, keep = ow[:-max_waits], ow[-max_waits:]
                    for k, w in enumerate(extra):
                        nop = mybir.InstNoOp(name=f"{inst.name}-w{k}", ins=[], outs=[])
                        nop.engine = inst.engine
                        nop.sync_info = mybir.SyncInfo(on_wait=[w], on_update=[])
                        new.append(nop)
                    inst.sync_info = mybir.SyncInfo(
                        on_wait=keep,
                        on_update=list(si.on_update) if si.on_update else [])
                    changed = True
                new.append(inst)
            if changed:
                b.instructions = new


def _build_nc():
    f32 = mybir.dt.float32
    bf16 = mybir.dt.bfloat16
    Alu = mybir.AluOpType
    Act = mybir.ActivationFunctionType

    nc = bass.Bass(trn_type="TRN2", target_bir_lowering=False, debug=False)
    qT = nc.dram_tensor("qT", [128, (N // CH) * KT * CH], bf16, kind="ExternalInput")
    labd = nc.dram_tensor("lab", [N, 1], f32, kind="ExternalInput")
    antid = nc.dram_tensor("anti", [128, 128], f32, kind="ExternalInput")
    identd = nc.dram_tensor("ident", [128, 128], f32, kind="ExternalInput")
    outd = nc.dram_tensor("out", [128, 2], f32, kind="ExternalOutput")

    with tile.TileContext(nc) as tc, contextlib.ExitStack() as ctx:
        qp = ctx.enter_context(tc.tile_pool(name="qp", bufs=1))
        pp = ctx.enter_context(tc.tile_pool(name="pp", bufs=2, space="PSUM"))
        ep = ctx.enter_context(tc.tile_pool(name="ep", bufs=3))
        wp = ctx.enter_context(tc.tile_pool(name="wp", bufs=2))
        sp = ctx.enter_context(tc.tile_pool(name="sp", bufs=1))

        # ---- preload ----
        # qT chunks: [128, KT, CH] bf16, one per 512-col chunk
        qt = []
        for n in range(N // CH):
            q = qp.tile([128, KT, CH], bf16, tag=f"q{n}")
            eng = nc.sync if n % 2 == 0 else nc.scalar
            eng.dma_start(
                out=q, in_=qT[:, n * KT * CH:(n + 1) * KT * CH])
            qt.append(q)
        # row labels per (partition, tile): lab[PAD + t*128 + p]
        lab_rows = sp.tile([128, NT, 1], f32)
        nc.sync.dma_start(
            out=lab_rows,
            in_=labd[PAD:PAD + R, :].rearrange("(t p) o -> p t o", p=128))
        # column labels broadcast to all partitions, cols [0, NT*128+WIN)
        labw_w = (NT - 1) * 128 + WIN        # 1536
        labw = sp.tile([128, labw_w], f32)
        nc.sync.dma_start(
            out=labw,
            in_=bass.AP(tensor=labd, offset=0, ap=[[0, 128], [1, labw_w]]))
        anti = sp.tile([128, 128], f32)
        nc.sync.dma_start(out=anti, in_=antid.ap())
        ident = sp.tile([128, 128], f32)
        nc.sync.dma_start(out=ident, in_=identd.ap())
        eps_t = sp.tile([128, 1], f32)
        nc.vector.memset(eps_t, EPS)
        warm = sp.tile([128, 128], bf16)
        nc.vector.memset(warm, 0.0)
        warm_ps = pp.tile([128, GRP], f32, tag="ps")
        for w in range(48):
            nc.tensor.matmul(warm_ps[:, :128], warm, warm, start=True, stop=True)

        # ---- accumulators ----
        dacc = sp.tile([128, NT * NG], f32)   # exp row-sums per (t, g)
        nacc = sp.tile([128, NT], f32)        # numerator per t
        edacc = sp.tile([128, NT], f32)       # diagonal exp per t

        # ---- main loop ----
        for t in range(NT):
            a = (PAD + t * 128) // CH        # lhsT chunk index
            off = (PAD + t * 128) % CH       # lhsT offset within chunk
            for g in range(NG):
                ps = pp.tile([128, GRP], f32, tag="ps")
                for sub in range(GRP // CH):
                    n = g * (GRP // CH) + sub
                    for k in range(KT):
                        nc.tensor.matmul(
                            ps[:, sub * CH:(sub + 1) * CH],
                            qt[a][:, k, off:off + 128],
                            qt[n][:, k, :],
                            start=(k == 0), stop=(k == KT - 1))
                e = ep.tile([128, GRP], f32, tag="e")
                nc.scalar.activation(
                    out=e, in_=ps[:], func=Act.Exp, scale=float(1.0 / TEMP),
                    accum_out=dacc[:, t * NG + g:t * NG + g + 1])
                if g == 0:
                    # window = cols [t*128, t*128+WIN) -- inside group 0
                    w0 = t * 128
                    u = wp.tile([128, WIN], f32, tag="u")
                    # u = (lab_col == lab_row) * exp(sim)
                    nc.vector.scalar_tensor_tensor(
                        out=u, in0=labw[:, w0:w0 + WIN],
                        scalar=lab_rows[:, t, :], in1=e[:, w0:w0 + WIN],
                        op0=Alu.is_equal, op1=Alu.mult)
                    # diagonal sits at window cols [PAD, PAD+128)
                    scr = wp.tile([128, 128], f32, tag="scr")
                    nc.vector.scalar_tensor_tensor(
                        out=scr, in0=u[:, PAD:PAD + 128], scalar=1.0,
                        in1=ident, op0=Alu.mult, op1=Alu.mult,
                        accum_out=edacc[:, t:t + 1])
                    nc.vector.tensor_tensor(
                        out=u[:, PAD:PAD + 128], in0=u[:, PAD:PAD + 128],
                        in1=anti, op=Alu.mult)
                    # numerator: sum over u where u > 1  (sim>0 gate)
                    scr2 = wp.tile([128, WIN], f32, tag="scr2")
                    nc.vector.scalar_tensor_tensor(
                        out=scr2, in0=u, scalar=1.0, in1=u,
                        op0=Alu.is_gt, op1=Alu.mult,
                        accum_out=nacc[:, t:t + 1])

        # ---- epilogue (all [128, NT]) ----
        dred = sp.tile([128, NT], f32)
        nc.vector.tensor_reduce(
            out=dred, in_=dacc.rearrange("p (t g) -> p t g", g=NG),
            axis=mybir.AxisListType.X, op=Alu.add)
        den = sp.tile([128, NT], f32)
        nc.vector.tensor_tensor(out=den, in0=dred, in1=edacc, op=Alu.subtract)
        v1 = sp.tile([128, NT], f32)
        nc.vector.tensor_scalar(out=v1, in0=nacc, scalar1=0.0, scalar2=None,
                                op0=Alu.is_gt)
        v2 = sp.tile([128, NT], f32)
        nc.vector.tensor_scalar(out=v2, in0=den, scalar1=0.0, scalar2=None,
                                op0=Alu.is_gt)
        v = sp.tile([128, NT], f32)
        nc.vector.tensor_tensor(out=v, in0=v1, in1=v2, op=Alu.mult)
        inv = sp.tile([128, NT], f32)
        nc.vector.tensor_scalar(out=inv, in0=v, scalar1=0.0, scalar2=None,
                                op0=Alu.is_equal)
        nsafe = sp.tile([128, NT], f32)
        nc.vector.tensor_tensor(out=nsafe, in0=nacc, in1=v, op=Alu.mult)
        nc.vector.tensor_tensor(out=nsafe, in0=nsafe, in1=inv, op=Alu.add)
        dsafe = sp.tile([128, NT], f32)
        nc.vector.tensor_tensor(out=dsafe, in0=den, in1=v, op=Alu.mult)
        nc.vector.tensor_tensor(out=dsafe, in0=dsafe, in1=inv, op=Alu.add)
        lgd = sp.tile([128, NT], f32)
        nc.scalar.activation(out=lgd, in_=dsafe, func=Act.Ln, bias=eps_t[:], scale=1.0)
        lgn = sp.tile([128, NT], f32)
        nc.scalar.activation(out=lgn, in_=nsafe, func=Act.Ln, scale=1.0)
        li = sp.tile([128, NT], f32)
        nc.vector.tensor_tensor(out=li, in0=lgd, in1=lgn, op=Alu.subtract)
        nc.vector.tensor_tensor(out=li, in0=li, in1=v, op=Alu.mult)
        o = sp.tile([128, 2], f32)
        nc.vector.tensor_reduce(out=o[:, 0:1], in_=li, axis=mybir.AxisListType.X,
                                op=Alu.add)
        nc.vector.tensor_reduce(out=o[:, 1:2], in_=v, axis=mybir.AxisListType.X,
                                op=Alu.add)
        nc.sync.dma_start(out=outd.ap(), in_=o)

    _split_excess_waits(nc)
    return nc


_NC = None


def _get_nc():
    global _NC
    if _NC is None:
        _NC = _build_nc()
    return _NC


def _host_reference(emb, lab):
    """Numpy fallback (only for pathological label distributions where a
    class exceeds the PAD margin; never triggers for the target regime)."""
    e = emb / np.linalg.norm(emb, axis=1, keepdims=True).astype(np.float32)
    sim = (e @ e.T).astype(np.float32) / np.float32(TEMP)
    E = np.exp(sim, dtype=np.float32)
    pos = (lab[:, None] == lab[None, :]) & ~np.eye(len(lab), dtype=bool)
    valid = pos & (sim > 0)
    num = np.where(valid, E, 0).sum(1, dtype=np.float32)
    den = E.sum(1, dtype=np.float32) - np.diagonal(E)
    rv = valid.any(1) & (den > 0)
    ns = np.where(rv, num, np.float32(1.0))
    ds = np.where(rv, den, np.float32(1.0))
    li = np.log(ds + np.float32(EPS)) - np.log(ns)
    nv = int(rv.sum())
    if nv == 0:
        return np.float32(0.0)
    return np.float32(abs(float(np.where(rv, li, 0).sum(dtype=np.float64)) / nv))


def kernel(**inputs):
    global LAST_RESULTS
    emb = np.ascontiguousarray(np.asarray(inputs["embeddings"], dtype=np.float32))
    lab = np.asarray(inputs["labels"]).astype(np.int64).ravel()
    assert emb.shape == (N, D) and lab.shape == (N,)

    if np.bincount(lab, minlength=1).max() > PAD:
        return _host_reference(emb, lab)

    _install_axon_trace_hook()

    # host prep: normalize, sort by label, per-core roll + transpose
    e = emb / np.linalg.norm(emb, axis=1, keepdims=True).astype(np.float32)
    order = np.argsort(lab, kind="stable")
    es = np.ascontiguousarray(e[order])
    ls = lab[order].astype(np.float32)

    anti = (1.0 - np.eye(128, dtype=np.float32)).astype(np.float32)
    ident = np.eye(128, dtype=np.float32)

    in_maps = []
    for c in range(M):
        shift = c * R - PAD
        rolled = np.roll(es, -shift, axis=0)         # [N, D] f32
        labr = np.roll(ls, -shift).reshape(N, 1)     # [N, 1] f32
        # [D, N] -> [128, NCH, KT, CH]: partition p, chunk n holds
        # qT[k*128+p, n*CH:(n+1)*CH] contiguckus per (k)
        qTc = (rolled.T.reshape(KT, 128, N // CH, CH)
               .transpose(1, 2, 0, 3)
               .reshape(128, (N // CH) * KT * CH)
               .astype(ml_dtypes.bfloat16))
        qTc = np.ascontiguousarray(qTc)
        in_maps.append({
            "qT": qTc,
            "lab": np.ascontiguousarray(labr),
            "anti": anti,
            "ident": ident,
        })

    nc = _get_nc()
    res = run_bass_kernel_spmd(nc, in_maps, core_ids=list(range(M)))
    LAST_RESULTS = res

    loss_sum = 0.0
    cnt = 0.0
    for c in range(M):
        o = res.results[c]["out"]
        loss_sum += float(o[:, 0].sum(dtype=np.float64))
        cnt += float(o[:, 1].sum(dtype=np.float64))
    if cnt <= 0:
        return np.float32(0.0)
    return np.float32(abs(loss_sum / cnt))


# revision 14
# speedup vs baseline: 1.0084x; 1.0013x over previous
"""Contrastive-loss Trainium2 kernel: 8-way data-parallel over similarity rows.

Strategy (per sharding hint): each of the 8 NeuronCores computes a
[1024, 8192] block of the similarity matrix sim = e @ e.T / T against the
full embedding matrix, reduces per-row numerator / denominator / validity
on-device, and returns per-partition partial (loss_sum, valid_count); the
host sums the 8x[128,2] partials.

Key layout trick: rows are sorted by label on the host and each core's
input is rolled so its 1024 rows sit at a fixed offset (PAD). Same-label
columns of any 128-row tile then live in a fixed 640-wide window
[t*128, t*128+640), so the label-mask / positive-gate / numerator work
touches 640 instead of 8192 columns per row. The denominator row-sum comes
free from the Exp activation's accum_out. Matmuls run in bf16 (fp32 PSUM
accumulate); everything downstream of exp is fp32.
"""

import contextlib
import ctypes
import os
import sys
import types

import ml_dtypes
import numpy as np

import concourse.bass as bass
import concourse.mybir as mybir
import concourse.tile as tile
from concourse.bass_utils import run_bass_kernel_spmd

# problem constants (hardcoded per task contract)
N, D, NCLS = 8192, 512, 512
TEMP = 0.07
EPS = 1e-8
M = 8            # cores
R = N // M       # 1024 rows per core
NT = R // 128    # 8 row-tiles per core
PAD = 256        # roll margin; must exceed max class size
WIN = 128 + 2 * PAD   # 640 col window containing all same-label cols of a tile
CH = 512         # matmul moving-dim chunk (one PSUM bank)
GRP = 2048       # columns per psum group / exp call (4 banks)
NG = N // GRP    # 4 groups
KT = D // 128    # 4 contraction tiles

_AXON_SO = "/opt/axon/libaxon_pjrt.so"

LAST_RESULTS = None   # BassKernelResults of the most recent run (for test.py)


def _install_axon_trace_hook():
    """Provide antenv.axon_hooks (NTFF profiling) if the image lacks it."""
    try:
        from antenv.axon_hooks import get_axon_ntff_profile_hook  # noqa: F401
        return
    except ImportError:
        pass
    if not os.path.exists(_AXON_SO):
        return
    try:
        lib = ctypes.CDLL(_AXON_SO)
    except OSError:
        return
    if not hasattr(lib, "axon_start_nrt_profile"):
        return
    lib.axon_start_nrt_profile.argtypes = [ctypes.POINTER(ctypes.c_int64), ctypes.c_size_t]
    lib.axon_start_nrt_profile.restype = ctypes.c_int64
    lib.axon_stop_nrt_profile.argtypes = [ctypes.c_char_p]
    lib.axon_stop_nrt_profile.restype = ctypes.c_int64

    @contextlib.contextmanager
    def _hook(output_dir, device_ids):
        import jax
        jax.devices()
        if device_ids:
            ids = (ctypes.c_int64 * len(device_ids))(*device_ids)
            rc = lib.axon_start_nrt_profile(ids, len(device_ids))
        else:
            rc = lib.axon_start_nrt_profile(None, 0)
        if rc != 0:
            raise RuntimeError(f"axon_start_nrt_profile rc={rc}")
        try:
            yield
        finally:
            n = lib.axon_stop_nrt_profile(str(output_dir).encode())
            if n < 0:
                raise RuntimeError(f"axon_stop_nrt_profile rc={n}")

    _the_hook = [_hook]
    mod = types.ModuleType("antenv.axon_hooks")
    mod.set_axon_ntff_profile_hook = lambda h: _the_hook.__setitem__(0, h)
    mod.get_axon_ntff_profile_hook = lambda: _the_hook[0]
    sys.modules["antenv.axon_hooks"] = mod
    import antenv
    antenv.axon_hooks = mod


def _split_excess_waits(nc, max_waits=1):
    """This walrus build allows one sync-wait per instruction; move extras
    onto same-engine NoOps inserted just before (execution order preserved)."""
    for f in nc.m.functions:
        for b in f.blocks:
            insts = b.instructions
            new = []
            changed = False
            for inst in insts:
                si = inst.sync_info
                ow = list(si.on_wait) if (si and si.on_wait) else []
                if len(ow) > max_waits:
                    extra, keep = ow[:-max_waits], ow[-max_waits:]
                    for k, w in enumerate(extra):
                        nop = mybir.InstNoOp(name=f"{inst.name}-w{k}", ins=[], outs=[])
                        nop.engine = inst.engine
                        nop.sync_info = mybir.SyncInfo(on_wait=[w], on_update=[])
                        new.append(nop)
                    inst.sync_info = mybir.SyncInfo(
                        on_wait=keep,
                        on_update=list(si.on_update) if si.on_update else [])
                    changed = True
                new.append(inst)
            if changed:
                b.instructions = new


def _build_nc():
    f32 = mybir.dt.float32
    bf16 = mybir.dt.bfloat16
    Alu = mybir.AluOpType
    Act = mybir.ActivationFunctionType

    nc = bass.Bass(trn_type="TRN2", target_bir_lowering=False, debug=False)
    qT = nc.dram_tensor("qT", [128, (N // CH) * KT * CH], bf16, kind="ExternalInput")
    labd = nc.dram_tensor("lab", [N, 1], f32, kind="ExternalInput")
    antid = nc.dram_tensor("anti", [128, 128], f32, kind="ExternalInput")
    identd = nc.dram_tensor("ident", [128, 128], f32, kind="ExternalInput")
    outd = nc.dram_tensor("out", [128, 2], f32, kind="ExternalOutput")

    with tile.TileContext(nc) as tc, contextlib.ExitStack() as ctx:
        qp = ctx.enter_context(tc.tile_pool(name="qp", bufs=1))
        pp = ctx.enter_context(tc.tile_pool(name="pp", bufs=2, space="PSUM"))
        ep = ctx.enter_context(tc.tile_pool(name="ep", bufs=3))
        wp = ctx.enter_context(tc.tile_pool(name="wp", bufs=2))
        sp = ctx.enter_context(tc.tile_pool(name="sp", bufs=1))

        # ---- preload ----
        # qT chunks: [128, KT, CH] bf16, one per 512-col chunk
        qt = []
        for n in range(N // CH):
            q = qp.tile([128, KT, CH], bf16, tag=f"q{n}")
            nc.sync.dma_start(
                out=q, in_=qT[:, n * KT * CH:(n + 1) * KT * CH])
            qt.append(q)
        # row labels per (partition, tile): lab[PAD + t*128 + p]
        lab_rows = sp.tile([128, NT, 1], f32)
        nc.sync.dma_start(
            out=lab_rows,
            in_=labd[PAD:PAD + R, :].rearrange("(t p) o -> p t o", p=128))
        # column labels broadcast to all partitions, cols [0, NT*128+WIN)
        labw_w = (NT - 1) * 128 + WIN        # 1536
        labw = sp.tile([128, labw_w], f32)
        nc.sync.dma_start(
            out=labw,
            in_=bass.AP(tensor=labd, offset=0, ap=[[0, 128], [1, labw_w]]))
        anti = sp.tile([128, 128], f32)
        nc.sync.dma_start(out=anti, in_=antid.ap())
        ident = sp.tile([128, 128], f32)
        nc.sync.dma_start(out=ident, in_=identd.ap())
        eps_t = sp.tile([128, 1], f32)
        nc.vector.memset(eps_t, EPS)
        warm = sp.tile([128, 128], bf16)
        nc.vector.memset(warm, 0.0)
        warm_ps = pp.tile([128, GRP], f32, tag="ps")
        for w in range(48):
            nc.tensor.matmul(warm_ps[:, :128], warm, warm, start=True, stop=True)

        # ---- accumulators ----
        dacc = sp.tile([128, NT * NG], f32)   # exp row-sums per (t, g)
        nacc = sp.tile([128, NT], f32)        # numerator per t
        edacc = sp.tile([128, NT], f32)       # diagonal exp per t

        # ---- main loop ----
        for t in range(NT):
            a = (PAD + t * 128) // CH        # lhsT chunk index
            off = (PAD + t * 128) % CH       # lhsT offset within chunk
            for g in range(NG):
                ps = pp.tile([128, GRP], f32, tag="ps")
                for sub in range(GRP // CH):
                    n = g * (GRP // CH) + sub
                    for k in range(KT):
                        nc.tensor.matmul(
                            ps[:, sub * CH:(sub + 1) * CH],
                            qt[a][:, k, off:off + 128],
                            qt[n][:, k, :],
                            start=(k == 0), stop=(k == KT - 1))
                e = ep.tile([128, GRP], f32, tag="e")
                nc.scalar.activation(
                    out=e, in_=ps[:], func=Act.Exp, scale=float(1.0 / TEMP),
                    accum_out=dacc[:, t * NG + g:t * NG + g + 1])
                if g == 0:
                    # window = cols [t*128, t*128+WIN) -- inside group 0
                    w0 = t * 128
                    u = wp.tile([128, WIN], f32, tag="u")
                    # u = (lab_col == lab_row) * exp(sim)
                    nc.vector.scalar_tensor_tensor(
                        out=u, in0=labw[:, w0:w0 + WIN],
                        scalar=lab_rows[:, t, :], in1=e[:, w0:w0 + WIN],
                        op0=Alu.is_equal, op1=Alu.mult)
                    # diagonal sits at window cols [PAD, PAD+128)
                    scr = wp.tile([128, 128], f32, tag="scr")
                    nc.vector.scalar_tensor_tensor(
                        out=scr, in0=u[:, PAD:PAD + 128], scalar=1.0,
                        in1=ident, op0=Alu.mult, op1=Alu.mult,
                        accum_out=edacc[:, t:t + 1])
                    nc.vector.tensor_tensor(
                        out=u[:, PAD:PAD + 128], in0=u[:, PAD:PAD + 128],
                        in1=anti, op=Alu.mult)
                    # numerator: sum over u where u > 1  (sim>0 gate)
                    scr2 = wp.tile([128, WIN], f32, tag="scr2")
                    nc.vector.scalar_tensor_tensor(
                        out=scr2, in0=u, scalar=1.0, in1=u,
                        op0=Alu.is_gt, op1=Alu.mult,
                        accum_out=nacc[:, t:t + 1])

        # ---- epilogue (all [128, NT]) ----
        dred = sp.tile([128, NT], f32)
        nc.vector.tensor_reduce(
            out=dred, in_=dacc.rearrange("p (t g) -> p t g", g=NG),
            axis=mybir.AxisListType.X, op=Alu.add)
        den = sp.tile([128, NT], f32)
        nc.vector.tensor_tensor(out=den, in0=dred, in1=edacc, op=Alu.subtract)
        v1 = sp.tile([128, NT], f32)
        nc.vector.tensor_scalar(out=v1, in0=nacc, scalar1=0.0, scalar2=None,
                                op0=Alu.is_gt)
        v2 = sp.tile([128, NT], f32)
        nc.vector.tensor_scalar(out=v2, in0=den, scalar1=0.0, scalar2=None,
                                op0=Alu.is_gt)
        v = sp.tile([128, NT], f32)
        nc.vector.tensor_tensor(out=v, in0=v1, in1=v2, op=Alu.mult)
        inv = sp.tile([128, NT], f32)
        nc.vector.tensor_scalar(out=inv, in0=v, scalar1=0.0, scalar2=None,
                                op0=Alu.is_equal)
        nsafe = sp.tile([128, NT], f32)
        nc.vector.tensor_tensor(out=nsafe, in0=nacc, in1=v, op=Alu.mult)
        nc.vector.tensor_tensor(out=nsafe, in0=nsafe, in1=inv, op=Alu.add)
        dsafe = sp.tile([128, NT], f32)
        nc.vector.tensor_tensor(out=dsafe, in0=den, in1=v, op=Alu.mult)
        nc.vector.tensor_tensor(out=dsafe, in0=dsafe, in1=inv, op=Alu.add)
        lgd = sp.tile([128, NT], f32)
        nc.scalar.activation(out=lgd, in_=dsafe, func=Act.Ln, bias=eps_t[:], scale=1.0)
        lgn = sp.tile([128, NT], f32)
        nc.scalar.activation(out=lgn, in_=nsafe, func=Act.Ln, scale=1.0)
        li = sp.tile([128, NT], f32)
        nc.vector.tensor_tensor(out=li, in0=lgd, in1=lgn, op=Alu.subtract)
        nc.vector.tensor_tensor(out=li, in0=li, in1=v, op=Alu.mult)
        o = sp.tile([128, 2], f32)
        nc.vector.tensor_reduce(out=o[:, 0:1], in_=li, axis=mybir.AxisListType.X,
                                op=Alu.add)
        nc.vector.tensor_reduce(out=o[:, 1:2], in_=v, axis=mybir.AxisListType.X,
                                op=Alu.add)
        nc.sync.dma_start(out=outd.ap(), in_=o)

    _split_excess_waits(nc)
    return nc


_NC = None


def _get_nc():
    global _NC
    if _NC is None:
        _NC = _build_nc()
    return _NC


def _host_reference(emb, lab):
    """Numpy fallback (only for pathological label distributions where a
    class exceeds the PAD margin; never triggers for the target regime)."""
    e = emb / np.linalg.norm(emb, axis=1, keepdims=True).astype(np.float32)
    sim = (e @ e.T).astype(np.float32) / np.float32(TEMP)
    E = np.exp(sim, dtype=np.float32)
    pos = (lab[:, None] == lab[None, :]) & ~np.eye(len(lab), dtype=bool)
    valid = pos & (sim > 0)
    num = np.where(valid, E, 0).sum(1, dtype=np.float32)
    den = E.sum(1, dtype=np.float32) - np.diagonal(E)
    rv = valid.any(1) & (den > 0)
    ns = np.where(rv, num, np.float32(1.0))
    ds = np.where(rv, den, np.float32(1.0))
    li = np.log(ds + np.float32(EPS)) - np.log(ns)
    nv = int(rv.sum())
    if nv == 0:
        return np.float32(0.0)
    return np.float32(abs(float(np.where(rv, li, 0).sum(dtype=np.float64)) / nv))


def kernel(**inputs):
    global LAST_RESULTS
    emb = np.ascontiguousarray(np.asarray(inputs["embeddings"], dtype=np.float32))
    lab = np.asarray(inputs["labels"]).astype(np.int64).ravel()
    assert emb.shape == (N, D) and lab.shape == (N,)

    if np.bincount(lab, minlength=1).max() > PAD:
        return _host_reference(emb, lab)

    _install_axon_trace_hook()

    # host prep: normalize, sort by label, per-core roll + transpose
    e = emb / np.linalg.norm(emb, axis=1, keepdims=True).astype(np.float32)
    order = np.argsort(lab, kind="stable")
    es = np.ascontiguousarray(e[order])
    ls = lab[order].astype(np.float32)

    anti = (1.0 - np.eye(128, dtype=np.float32)).astype(np.float32)
    ident = np.eye(128, dtype=np.float32)

    in_maps = []
    for c in range(M):
        shift = c * R - PAD
        rolled = np.roll(es, -shift, axis=0)         # [N, D] f32
        labr = np.roll(ls, -shift).reshape(N, 1)     # [N, 1] f32
        # [D, N] -> [128, NCH, KT, CH]: partition p, chunk n holds
        # qT[k*128+p, n*CH:(n+1)*CH] contiguckus per (k)
        qTc = (rolled.T.reshape(KT, 128, N // CH, CH)
               .transpose(1, 2, 0, 3)
               .reshape(128, (N // CH) * KT * CH)
               .astype(ml_dtypes.bfloat16))
        qTc = np.ascontiguousarray(qTc)
        in_maps.append({
            "qT": qTc,
            "lab": np.ascontiguousarray(labr),
            "anti": anti,
            "ident": ident,
        })

    nc = _get_nc()
    res = run_bass_kernel_spmd(nc, in_maps, core_ids=list(range(M)))
    LAST_RESULTS = res

    loss_sum = 0.0
    cnt = 0.0
    for c in range(M):
        o = res.results[c]["out"]
        loss_sum += float(o[:, 0].sum(dtype=np.float64))
        cnt += float(o[:, 1].sum(dtype=np.float64))
    if cnt <= 0:
        return np.float32(0.0)
    return np.float32(abs(loss_sum / cnt))
